# revision 20
# baseline (speedup 1.0000x reference)
"""CoMPT message-passing layer on 8 Trainium2 NeuronCores (Bass/Tile), v2.

Algorithm notes (verified numerically against the jax reference):
  * In the reference, `agg = segment_sum(score * v[dst], dst)` - v[dst] is
    constant within each dst-segment, so agg[n] = (sum of scores into n) * v[n].
    The per-edge v gather disappears entirely.
  * Softmax max-subtraction is skipped (logits are O(1); pure rounding change).
  * Per-edge normalization folds into per-node sums:
        S[n,h] = sum_i t_i[n,h] / (s_i[n,h] + eps)
    where s_i = segsum(exp(l_i)), t_i = segsum(exp(l_i) * atten).

Distribution: edges are sorted by dst on the host and split across 8 cores at
node boundaries (contiguous dst-range per core); segment reductions are fully
core-local.  Per core the edge stream is packed into blocks of <=2048 edge
slots covering <=128 consecutive dst nodes.

v2 (vs the v1 baseline): no on-device q table / gpsimd dma_gather (which was
the measured bottleneck at ~63% engine-active).  Instead the host gathers
h_node[src] and h_node[dst] into edge-ordered streams and the device projects
q_src/q_dst per edge with the same stationary Wq matmul used for k.  Logits
are produced edge-major via per-chunk stationary matmuls (lhsT = product
chunk, rhs = head-mask), which eliminates all PE transposes in the edge phase.
The segment-sum one-hot is stored fp8 (exact 0/1) to halve its DMA cost.
"""

import numpy as np
import ml_dtypes

import concourse.bass as bass
import concourse.mybir as mybir
import concourse.tile as tile
from concourse import bacc
from concourse import bass_utils
from concourse.bass import ts
from concourse.masks import make_identity

# ---------------------------------------------------------------- constants
N = 50000
E = 800000
D = 128
H = 8
DH = 16
NCORES = 8
P = 128

CHUNK = 128           # edges per chunk (one one-hot / logits matmul)
CBLK = 16             # chunks per block
BE = CHUNK * CBLK     # 2048 edge slots per block
TE = 512              # edges per pipeline tile
TPB = BE // TE        # tiles per block (4)
EPS = 1e-12

BF16 = mybir.dt.bfloat16
F32 = mybir.dt.float32
FP8 = mybir.dt.float8e4
AF = mybir.ActivationFunctionType
OP = mybir.AluOpType

_nc_cache = {}


# ---------------------------------------------------------------- host prep
def _prep(h_node, h_edge, distance, Wq, bq, Wk, bk, Wv, bv, Wo, bo, lam,
          src, dst):
    """Sort/shard/pack on the host. Returns (cfg, in_maps, meta)."""
    n = h_node.shape[0]
    e = h_edge.shape[0]

    order = np.argsort(dst, kind="stable")
    deg = np.bincount(dst, minlength=n).astype(np.int64)
    cum = np.concatenate([[0], np.cumsum(deg)])  # cum[i] = edges with dst < i

    # core cuts at node granularity, balancing edges
    targets = [(c * e) // NCORES for c in range(1, NCORES)]
    cuts = [0] + [int(np.searchsorted(cum, t)) for t in targets] + [n]

    lam_v = float(np.asarray(lam).reshape(-1)[0])
    atten = (distance.astype(np.float32) ** np.float32(lam_v)).astype(np.float32)

    # greedy block packing: consecutive nodes, <=128 nodes and <=BE edges
    core_blocks = []
    for c in range(NCORES):
        nlo, nhi = cuts[c], cuts[c + 1]
        blocks = []
        ns = nlo
        while ns < nhi:
            cnt = 0
            while (ns + cnt < nhi and cnt < P
                   and cum[ns + cnt + 1] - cum[ns] <= BE):
                cnt += 1
            assert cnt > 0, "node degree exceeds block capacity"
            blocks.append((ns, cnt, int(cum[ns]), int(cum[ns + cnt])))
            ns += cnt
        core_blocks.append(blocks)

    nblk = max(len(b) for b in core_blocks)
    ep = nblk * BE
    g = ep // CHUNK

    h_node_f8 = h_node.astype(ml_dtypes.float8_e4m3)
    h_edge_f8 = h_edge.astype(ml_dtypes.float8_e4m3)

    w_common = {
        "rhs_q": np.ascontiguousarray(Wq.T).astype(ml_dtypes.bfloat16),
        "lhs_k": np.ascontiguousarray(Wk.T).astype(ml_dtypes.bfloat16),
        "rhs_v": np.ascontiguousarray(Wv.T).astype(np.float32),
        "rhs_o": np.ascontiguousarray(Wo.T).astype(np.float32),
        "mh8": np.kron(np.eye(H), np.ones((DH, 1))).astype(ml_dtypes.bfloat16),
        "bqc": np.ascontiguousarray(bq.reshape(P, 1)).astype(np.float32),
        "bkc": np.ascontiguousarray(bk.reshape(P, 1)).astype(np.float32),
        "bvr": np.ascontiguousarray(bv.reshape(1, P)).astype(np.float32),
        "bor": np.ascontiguousarray(bo.reshape(1, P)).astype(np.float32),
    }

    in_maps = []
    meta = []
    for c in range(NCORES):
        blocks = core_blocks[c]
        heT = np.zeros((P, ep), ml_dtypes.float8_e4m3)
        hsT = np.zeros((P, ep), ml_dtypes.float8_e4m3)
        hdT = np.zeros((P, ep), ml_dtypes.float8_e4m3)
        u8 = np.zeros((P, nblk, CBLK, P), ml_dtypes.float8_e4m3)
        distT = np.zeros((P, g), np.float32)
        hTb = np.zeros((P, nblk * P), np.float32)

        for b, (ns, cnt, elo, ehi) in enumerate(blocks):
            ids = order[elo:ehi]
            ne = len(ids)
            sl = slice(b * BE, b * BE + ne)
            heT[:, sl] = h_edge_f8[ids].T
            hsT[:, sl] = h_node_f8[src[ids]].T
            hdT[:, sl] = h_node_f8[dst[ids]].T
            pos = np.arange(ne)
            loc = dst[ids] - ns
            u8[pos % P, b, pos // P, loc] = 1.0
            distT[pos % P, b * CBLK + pos // P] = atten[ids]
            hTb[:, b * P:b * P + cnt] = h_node[ns:ns + cnt].T

        in_maps.append({
            "heT": heT,
            "hsT": hsT,
            "hdT": hdT,
            "u8": np.ascontiguousarray(u8.reshape(P, nblk * CBLK * P)),
            "distT": distT,
            "hTb": hTb,
            **w_common,
        })
        meta.append(blocks)

    cfg = dict(nblk=nblk, n=n, lam=lam_v, use_eps=bool((deg[:n] == 0).any()),
               use_bq=bool(np.any(bq)), use_bk=bool(np.any(bk)),
               use_bv=bool(np.any(bv)), use_bo=bool(np.any(bo)))
    return cfg, in_maps, meta


# ---------------------------------------------------------------- builder
def build_program(cfg):
    nblk = cfg["nblk"]
    ep = nblk * BE
    g = ep // CHUNK

    nc = bacc.Bacc("TRN2", target_bir_lowering=False, debug=False,
                   num_devices=NCORES)

    heT_d = nc.dram_tensor("heT", [P, ep], FP8, kind="ExternalInput").ap()
    hsT_d = nc.dram_tensor("hsT", [P, ep], FP8, kind="ExternalInput").ap()
    hdT_d = nc.dram_tensor("hdT", [P, ep], FP8, kind="ExternalInput").ap()
    u8_d = nc.dram_tensor("u8", [P, nblk * CBLK * P], FP8, kind="ExternalInput").ap()
    distT_d = nc.dram_tensor("distT", [P, g], F32, kind="ExternalInput").ap()
    hTb_d = nc.dram_tensor("hTb", [P, nblk * P], F32, kind="ExternalInput").ap()
    rhs_q_d = nc.dram_tensor("rhs_q", [P, P], BF16, kind="ExternalInput").ap()
    lhs_k_d = nc.dram_tensor("lhs_k", [P, P], BF16, kind="ExternalInput").ap()
    rhs_v_d = nc.dram_tensor("rhs_v", [P, P], F32, kind="ExternalInput").ap()
    rhs_o_d = nc.dram_tensor("rhs_o", [P, P], F32, kind="ExternalInput").ap()
    mh8_d = nc.dram_tensor("mh8", [P, H], BF16, kind="ExternalInput").ap()
    bqc_d = nc.dram_tensor("bqc", [P, 1], F32, kind="ExternalInput").ap()
    bkc_d = nc.dram_tensor("bkc", [P, 1], F32, kind="ExternalInput").ap()
    bvr_d = nc.dram_tensor("bvr", [1, P], F32, kind="ExternalInput").ap()
    bor_d = nc.dram_tensor("bor", [1, P], F32, kind="ExternalInput").ap()
    out_d = nc.dram_tensor("out", [nblk * P, P], F32, kind="ExternalOutput").ap()

    def bcast(ap, inner):
        return bass.AP(tensor=ap.tensor, offset=ap.offset, ap=ap.ap + [[0, inner]])

    from contextlib import ExitStack
    with tile.TileContext(nc) as tc, ExitStack() as stk:
        const = stk.enter_context(tc.tile_pool(name="const", bufs=1))

        rhs_q = const.tile([P, P], BF16); nc.sync.dma_start(rhs_q[:], rhs_q_d[:, :])
        lhs_k = const.tile([P, P], BF16); nc.sync.dma_start(lhs_k[:], lhs_k_d[:, :])
        rhs_v = const.tile([P, P], F32); nc.sync.dma_start(rhs_v[:], rhs_v_d[:, :])
        rhs_o = const.tile([P, P], F32); nc.sync.dma_start(rhs_o[:], rhs_o_d[:, :])
        mh8 = const.tile([P, H], BF16); nc.sync.dma_start(mh8[:], mh8_d[:, :])
        bqc = const.tile([P, 1], F32); nc.sync.dma_start(bqc[:], bqc_d[:, :])
        bkc = const.tile([P, 1], F32); nc.sync.dma_start(bkc[:], bkc_d[:, :])
        brow = {}
        for nm, ap_d in (("bvr", bvr_d), ("bor", bor_d)):
            brow[nm] = const.tile([P, P], F32, name=f"brow_{nm}")
            rep = bass.AP(tensor=ap_d.tensor, offset=ap_d[:, :].offset,
                          ap=[[0, P]] + ap_d[:, :].ap[1:])
            nc.sync.dma_start(brow[nm][:], rep)

        id_f = const.tile([P, P], F32); make_identity(nc, id_f[:])
        distT = const.tile([P, g], F32); nc.sync.dma_start(distT[:], distT_d[:, :])
        hTb_all = const.tile([P, nblk, P], F32)

        x_all = const.tile([P, nblk, P], F32)   # pre-mish outputs

        # ---------------- unified edge + output loop ----------------
        # PSUM: qdps 2 banks + kps 1 + lps 1 + sps 1 + v/aggt/o 3 = 8.
        # Per-block output work (normalize, v/o projections) is interleaved
        # into the edge loop so no phase barrier serializes the program;
        # mish + store go in groups of GS blocks.
        s8_all = const.tile([P, nblk, H], F32)
        GS = 10
        xb = stk.enter_context(tc.tile_pool(name="xb", bufs=4))
        with tc.tile_pool(name="eb", bufs=4) as eb, \
             tc.tile_pool(name="et", bufs=6) as et, \
             tc.tile_pool(name="qdps", bufs=2, space="PSUM") as qdps, \
             tc.tile_pool(name="kps", bufs=2, space="PSUM") as kps, \
             tc.tile_pool(name="lps", bufs=1, space="PSUM") as lps, \
             tc.tile_pool(name="sps", bufs=1, space="PSUM") as sps:
            for b in range(nblk):
                he_b = eb.tile([P, BE], FP8, tag="he")
                nc.sync.dma_start(he_b[:], heT_d[:, ts(b, BE)])
                hs_b = eb.tile([P, BE], FP8, tag="hs")
                nc.sync.dma_start(hs_b[:], hsT_d[:, ts(b, BE)])
                hd_b = eb.tile([P, BE], FP8, tag="hd")
                nc.sync.dma_start(hd_b[:], hdT_d[:, ts(b, BE)])
                u8_b = eb.tile([P, CBLK, P], FP8, tag="u8")
                nc.sync.dma_start(u8_b[:], u8_d[:, ts(b, CBLK * P)])
                ps_s = sps.tile([P, 48], F32, tag="s")
                # projections for tile 0 up front; tiles 1..3 are emitted
                # interleaved into the previous tile's logit matmuls so their
                # LDWEIGHTS hide under the long projection streams
                proj = {}

                def emit_proj(t):
                    qd_ps = qdps.tile([P, 2, TE], F32, tag="qd")
                    nc.tensor.matmul(qd_ps[:, 0, :], rhs_q, hs_b[:, ts(t, TE)])
                    nc.tensor.matmul(qd_ps[:, 1, :], rhs_q, hd_b[:, ts(t, TE)])
                    k_ps = kps.tile([P, TE], F32, tag="k")
                    nc.tensor.matmul(k_ps[:], lhs_k, he_b[:, ts(t, TE)])
                    proj[t] = (qd_ps, k_ps)

                emit_proj(0)
                for t in range(TPB):
                    qd_ps, k_ps = proj.pop(t)

                    qsd = et.tile([P, 2, TE], BF16, tag="qsd")
                    if cfg.get("use_bq"):
                        nc.scalar.activation(qsd[:], qd_ps[:], AF.Identity,
                                             bias=bqc[:, :1])
                    else:
                        nc.scalar.copy(qsd[:], qd_ps[:])

                    # prod0/prod1 = (qs,qd) * k in one op, k read straight
                    # from PSUM with a step-0 broadcast over the pair dim
                    prod = et.tile([P, 3, TE], BF16, tag="prod")
                    if cfg.get("use_bk"):
                        kt = et.tile([P, TE], BF16, tag="kt")
                        nc.scalar.activation(kt[:], k_ps[:], AF.Identity,
                                             bias=bkc[:, :1])
                        k_src = bass.AP(tensor=kt[:].tensor, offset=kt[:].offset,
                                        ap=kt[:].ap[:1] + [[0, 2]] + kt[:].ap[1:])
                    else:
                        k_src = bass.AP(tensor=k_ps[:].tensor, offset=k_ps[:].offset,
                                        ap=k_ps[:].ap[:1] + [[0, 2]] + k_ps[:].ap[1:])
                    nc.vector.tensor_tensor(prod[:, 0:2, :], qsd[:], k_src,
                                            op=OP.mult)
                    nc.gpsimd.tensor_mul(prod[:, 2, :], qsd[:, 0, :], qsd[:, 1, :])

                    # edge-major logits: per 128-edge chunk, stationary = prod;
                    # next tile's projection matmuls are interleaved to hide
                    # the per-chunk LDWEIGHTS under their 512-wide streams
                    ps_l = lps.tile([P, TE // P, 24], F32, tag="l")
                    nxt = []
                    if t + 1 < TPB:
                        qd_n = qdps.tile([P, 2, TE], F32, tag="qd")
                        k_n = kps.tile([P, TE], F32, tag="k")
                        nxt = [lambda: nc.tensor.matmul(qd_n[:, 0, :], rhs_q,
                                                        hs_b[:, ts(t + 1, TE)]),
                               lambda: nc.tensor.matmul(qd_n[:, 1, :], rhs_q,
                                                        hd_b[:, ts(t + 1, TE)]),
                               lambda: nc.tensor.matmul(k_n[:], lhs_k,
                                                        he_b[:, ts(t + 1, TE)])]
                        proj[t + 1] = (qd_n, k_n)
                    for cc in range(TE // P):
                        for j in range(3):
                            nc.tensor.matmul(ps_l[:, cc, 8 * j:8 * (j + 1)],
                                             prod[:, j, ts(cc, P)], mh8[:])
                        if cc < len(nxt):
                            nxt[cc]()

                    # exp and attenuation, edge-major
                    xs = et.tile([P, TE // P, 48], BF16, tag="xs")
                    xs_lo = bass.AP(tensor=xs[:].tensor, offset=xs[:].offset,
                                    ap=xs[:].ap[:1] + [[48, TE // P], [1, 24]])
                    nc.scalar.activation(xs_lo, ps_l[:], AF.Exp, scale=0.25)
                    tg = b * CBLK + t * (TE // P)
                    att = bass.AP(tensor=distT.tensor,
                                  offset=distT[:, tg:tg + TE // P].offset,
                                  ap=distT[:].ap[:1] + [[1, TE // P], [0, 24]])
                    xs_hi = bass.AP(tensor=xs[:].tensor, offset=xs[:].offset + 24,
                                    ap=xs[:].ap[:1] + [[48, TE // P], [1, 24]])
                    nc.vector.tensor_tensor(xs_hi, xs_lo, att, op=OP.mult)

                    # segment sums: one-hot matmul accumulated over the block
                    for cc in range(TE // P):
                        lc = t * (TE // P) + cc
                        nc.tensor.matmul(ps_s[:], u8_b[:, lc, :], xs[:, cc, :],
                                         start=(lc == 0), stop=(lc == CBLK - 1))

                # ---- per-block output work, interleaved ----
                # S[n,h] = sum_t t_sums/(s_sums+eps): reciprocal on ACT,
                # small combines on DVE
                rcp = xb.tile([P, 24], F32, tag="rcp")
                if cfg.get("use_eps"):
                    nc.vector.tensor_scalar_add(rcp[:], ps_s[:, 0:24], EPS)
                    nc.vector.reciprocal(rcp[:], rcp[:])
                else:
                    nc.vector.reciprocal(rcp[:], ps_s[:, 0:24])
                m24 = xb.tile([P, 24], F32, tag="m24")
                nc.vector.tensor_mul(m24[:], ps_s[:, 24:48], rcp[:])
                m24v = bass.AP(tensor=m24[:].tensor, offset=m24[:].offset,
                               ap=[m24[:].ap[0], [1, H], [H, 3]])
                nc.vector.tensor_reduce(s8_all[:, b, :], m24v,
                                        axis=mybir.AxisListType.X, op=OP.add)

        nc.sync.dma_start(hTb_all[:], hTb_d[:, :])
        with tc.tile_pool(name="fb", bufs=6) as fb, \
             tc.tile_pool(name="fps", bufs=2, space="PSUM") as fps:
            for b in range(nblk):
                v_ps = fps.tile([P, 512], F32, tag="v")
                nc.tensor.matmul(v_ps[:, 0:P], hTb_all[:, b, :], rhs_v[:])
                agg = fb.tile([P, P], F32, tag="agg")
                a3 = agg[:].rearrange("p (h d) -> p h d", h=H)
                if cfg.get("use_bv"):
                    v_sb = fb.tile([P, P], F32, tag="vsb")
                    nc.scalar.copy(v_sb[:], v_ps[:, 0:P])
                    nc.vector.tensor_tensor(v_sb[:], v_sb[:], brow["bvr"][:, :],
                                            op=OP.add)
                    v3 = v_sb[:].rearrange("p (h d) -> p h d", h=H)
                else:
                    v3 = v_ps[:, 0:P].rearrange("p (h d) -> p h d", h=H)
                nc.vector.tensor_tensor(a3, v3, bcast(s8_all[:, b, :], DH),
                                        op=OP.mult)

                aggT_ps = fps.tile([P, 512], F32, tag="aggt")
                nc.tensor.transpose(aggT_ps[:, 0:P], agg[:], id_f[:])
                aggT = fb.tile([P, P], F32, tag="aggts")
                nc.scalar.copy(aggT[:], aggT_ps[:, 0:P])

                o_ps = fps.tile([P, 512], F32, tag="o")
                nc.tensor.matmul(o_ps[:, 0:P], aggT[:], rhs_o[:])
                nc.vector.tensor_copy(x_all[:, b, :], o_ps[:, 0:P])
                if cfg.get("use_bo"):
                    nc.vector.tensor_tensor(x_all[:, b, :], x_all[:, b, :],
                                            brow["bor"][:, :], op=OP.add)

            # ---- mish tail: mish(x) = x * tanh(ln(1 + e^x)), staged by
            # function across groups so the ACT table set switches only twice
            u_all = const.tile([P, nblk, P], F32, name="u_all")
            groups = [(g0, min(GS, nblk - g0)) for g0 in range(0, nblk, GS)]
            for g0, gs in groups:
                nc.scalar.activation(u_all[:, g0:g0 + gs, :],
                                     x_all[:, g0:g0 + gs, :], AF.Exp)
            for g0, gs in groups:
                nc.vector.tensor_scalar_add(u_all[:, g0:g0 + gs, :],
                                            u_all[:, g0:g0 + gs, :], 1.0)
            for g0, gs in groups:
                nc.scalar.activation(u_all[:, g0:g0 + gs, :],
                                     u_all[:, g0:g0 + gs, :], AF.Ln)
            for g0, gs in groups:
                nc.scalar.activation(u_all[:, g0:g0 + gs, :],
                                     u_all[:, g0:g0 + gs, :], AF.Tanh)
            for g0, gs in groups:
                o_g = fb.tile([P, GS, P], F32, tag="og")
                nc.vector.tensor_mul(o_g[:, :gs, :], x_all[:, g0:g0 + gs, :],
                                     u_all[:, g0:g0 + gs, :])
                dram = bass.AP(tensor=out_d.tensor, offset=g0 * P * P,
                               ap=[[P, P], [P * P, gs], [1, P]])
                nc.sync.dma_start(dram, o_g[:, :gs, :])

    nc.compile()
    return nc


# ---------------------------------------------------------------- entry
def kernel(**inputs):
    inputs = {k: np.asarray(v) for k, v in inputs.items()}
    cfg, in_maps, meta = _prep(**inputs)

    key = (cfg["nblk"], cfg["use_bq"], cfg["use_bk"], cfg["use_bv"],
           cfg["use_bo"])
    nc = _nc_cache.get(key)
    if nc is None:
        nc = build_program(cfg)
        _nc_cache[key] = nc

    res = bass_utils.run_bass_kernel_spmd(nc, in_maps,
                                          core_ids=list(range(NCORES)))

    n = cfg["n"]
    out = np.zeros((n, D), np.float32)
    for c in range(NCORES):
        oc = res.results[c]["out"]
        for b, (nstart, cnt, _, _) in enumerate(meta[c]):
            out[nstart:nstart + cnt] = oc[b * P:b * P + cnt]
    return out


# revision 21
# speedup vs baseline: 1.0614x; 1.0614x over previous
"""CoMPT message-passing layer on 8 Trainium2 NeuronCores (Bass/Tile), v2.

Algorithm notes (verified numerically against the jax reference):
  * In the reference, `agg = segment_sum(score * v[dst], dst)` - v[dst] is
    constant within each dst-segment, so agg[n] = (sum of scores into n) * v[n].
    The per-edge v gather disappears entirely.
  * Softmax max-subtraction is skipped (logits are O(1); pure rounding change).
  * Per-edge normalization folds into per-node sums:
        S[n,h] = sum_i t_i[n,h] / (s_i[n,h] + eps)
    where s_i = segsum(exp(l_i)), t_i = segsum(exp(l_i) * atten).

Distribution: edges are sorted by dst on the host and split across 8 cores at
node boundaries (contiguous dst-range per core); segment reductions are fully
core-local.  Per core the edge stream is packed into blocks of <=2048 edge
slots covering <=128 consecutive dst nodes.

v2 (vs the v1 baseline): no on-device q table / gpsimd dma_gather (which was
the measured bottleneck at ~63% engine-active).  Instead the host gathers
h_node[src] and h_node[dst] into edge-ordered streams and the device projects
q_src/q_dst per edge with the same stationary Wq matmul used for k.  Logits
are produced edge-major via per-chunk stationary matmuls (lhsT = product
chunk, rhs = head-mask), which eliminates all PE transposes in the edge phase.
The segment-sum one-hot is stored fp8 (exact 0/1) to halve its DMA cost.
"""

import numpy as np
import ml_dtypes

import concourse.bass as bass
import concourse.mybir as mybir
import concourse.tile as tile
from concourse import bacc
from concourse import bass_utils
from concourse.bass import ts
from concourse.masks import make_identity

# ---------------------------------------------------------------- constants
N = 50000
E = 800000
D = 128
H = 8
DH = 16
NCORES = 8
P = 128

CHUNK = 128           # edges per chunk (one one-hot / logits matmul)
CBLK = 16             # chunks per block
BE = CHUNK * CBLK     # 2048 edge slots per block
TE = 512              # edges per pipeline tile
TPB = BE // TE        # tiles per block (4)
EPS = 1e-12

BF16 = mybir.dt.bfloat16
F32 = mybir.dt.float32
FP8 = mybir.dt.float8e4
AF = mybir.ActivationFunctionType
OP = mybir.AluOpType

_nc_cache = {}


# ---------------------------------------------------------------- host prep
def _prep(h_node, h_edge, distance, Wq, bq, Wk, bk, Wv, bv, Wo, bo, lam,
          src, dst):
    """Sort/shard/pack on the host. Returns (cfg, in_maps, meta)."""
    n = h_node.shape[0]
    e = h_edge.shape[0]

    order = np.argsort(dst, kind="stable")
    deg = np.bincount(dst, minlength=n).astype(np.int64)
    cum = np.concatenate([[0], np.cumsum(deg)])  # cum[i] = edges with dst < i

    # core cuts at node granularity, balancing edges
    targets = [(c * e) // NCORES for c in range(1, NCORES)]
    cuts = [0] + [int(np.searchsorted(cum, t)) for t in targets] + [n]

    lam_v = float(np.asarray(lam).reshape(-1)[0])
    atten = (distance.astype(np.float32) ** np.float32(lam_v)).astype(np.float32)

    # greedy block packing: consecutive nodes, <=128 nodes and <=BE edges
    core_blocks = []
    for c in range(NCORES):
        nlo, nhi = cuts[c], cuts[c + 1]
        blocks = []
        ns = nlo
        while ns < nhi:
            cnt = 0
            while (ns + cnt < nhi and cnt < P
                   and cum[ns + cnt + 1] - cum[ns] <= BE):
                cnt += 1
            assert cnt > 0, "node degree exceeds block capacity"
            blocks.append((ns, cnt, int(cum[ns]), int(cum[ns + cnt])))
            ns += cnt
        core_blocks.append(blocks)

    nblk = max(len(b) for b in core_blocks)
    ep = nblk * BE
    g = ep // CHUNK

    h_node_f8 = h_node.astype(ml_dtypes.float8_e4m3)
    h_edge_f8 = h_edge.astype(ml_dtypes.float8_e4m3)

    w_common = {
        "rhs_q": np.ascontiguousarray(Wq.T).astype(ml_dtypes.bfloat16),
        "lhs_k": np.ascontiguousarray(Wk.T).astype(ml_dtypes.bfloat16),
        "rhs_v": np.ascontiguousarray(Wv.T).astype(np.float32),
        "rhs_o": np.ascontiguousarray(Wo.T).astype(np.float32),
        "mh8": np.kron(np.eye(H), np.ones((DH, 1))).astype(ml_dtypes.bfloat16),
        "bqc": np.ascontiguousarray(bq.reshape(P, 1)).astype(np.float32),
        "bkc": np.ascontiguousarray(bk.reshape(P, 1)).astype(np.float32),
        "bvr": np.ascontiguousarray(bv.reshape(1, P)).astype(np.float32),
        "bor": np.ascontiguousarray(bo.reshape(1, P)).astype(np.float32),
    }

    in_maps = []
    meta = []
    for c in range(NCORES):
        blocks = core_blocks[c]
        heT = np.zeros((P, ep), ml_dtypes.float8_e4m3)
        hsT = np.zeros((P, ep), ml_dtypes.float8_e4m3)
        hdT = np.zeros((P, ep), ml_dtypes.float8_e4m3)
        u8 = np.zeros((P, nblk, CBLK, P), ml_dtypes.float8_e4m3)
        distT = np.zeros((P, g), np.float32)
        hTb = np.zeros((P, nblk * P), np.float32)

        for b, (ns, cnt, elo, ehi) in enumerate(blocks):
            ids = order[elo:ehi]
            ne = len(ids)
            sl = slice(b * BE, b * BE + ne)
            heT[:, sl] = h_edge_f8[ids].T
            hsT[:, sl] = h_node_f8[src[ids]].T
            hdT[:, sl] = h_node_f8[dst[ids]].T
            pos = np.arange(ne)
            loc = dst[ids] - ns
            u8[pos % P, b, pos // P, loc] = 1.0
            distT[pos % P, b * CBLK + pos // P] = atten[ids]
            hTb[:, b * P:b * P + cnt] = h_node[ns:ns + cnt].T

        in_maps.append({
            "heT": heT,
            "hsT": hsT,
            "hdT": hdT,
            "u8": np.ascontiguousarray(u8.reshape(P, nblk * CBLK * P)),
            "distT": distT,
            "hTb": hTb,
            **w_common,
        })
        meta.append(blocks)

    cfg = dict(nblk=nblk, n=n, lam=lam_v, use_eps=bool((deg[:n] == 0).any()),
               use_bq=bool(np.any(bq)), use_bk=bool(np.any(bk)),
               use_bv=bool(np.any(bv)), use_bo=bool(np.any(bo)))
    return cfg, in_maps, meta


# ---------------------------------------------------------------- builder
def build_program(cfg):
    nblk = cfg["nblk"]
    ep = nblk * BE
    g = ep // CHUNK

    nc = bacc.Bacc("TRN2", target_bir_lowering=False, debug=False,
                   num_devices=NCORES)

    heT_d = nc.dram_tensor("heT", [P, ep], FP8, kind="ExternalInput").ap()
    hsT_d = nc.dram_tensor("hsT", [P, ep], FP8, kind="ExternalInput").ap()
    hdT_d = nc.dram_tensor("hdT", [P, ep], FP8, kind="ExternalInput").ap()
    u8_d = nc.dram_tensor("u8", [P, nblk * CBLK * P], FP8, kind="ExternalInput").ap()
    distT_d = nc.dram_tensor("distT", [P, g], F32, kind="ExternalInput").ap()
    hTb_d = nc.dram_tensor("hTb", [P, nblk * P], F32, kind="ExternalInput").ap()
    rhs_q_d = nc.dram_tensor("rhs_q", [P, P], BF16, kind="ExternalInput").ap()
    lhs_k_d = nc.dram_tensor("lhs_k", [P, P], BF16, kind="ExternalInput").ap()
    rhs_v_d = nc.dram_tensor("rhs_v", [P, P], F32, kind="ExternalInput").ap()
    rhs_o_d = nc.dram_tensor("rhs_o", [P, P], F32, kind="ExternalInput").ap()
    mh8_d = nc.dram_tensor("mh8", [P, H], BF16, kind="ExternalInput").ap()
    bqc_d = nc.dram_tensor("bqc", [P, 1], F32, kind="ExternalInput").ap()
    bkc_d = nc.dram_tensor("bkc", [P, 1], F32, kind="ExternalInput").ap()
    bvr_d = nc.dram_tensor("bvr", [1, P], F32, kind="ExternalInput").ap()
    bor_d = nc.dram_tensor("bor", [1, P], F32, kind="ExternalInput").ap()
    out_d = nc.dram_tensor("out", [nblk * P, P], F32, kind="ExternalOutput").ap()

    def bcast(ap, inner):
        return bass.AP(tensor=ap.tensor, offset=ap.offset, ap=ap.ap + [[0, inner]])

    from contextlib import ExitStack
    with tile.TileContext(nc) as tc, ExitStack() as stk:
        const = stk.enter_context(tc.tile_pool(name="const", bufs=1))

        rhs_q = const.tile([P, P], BF16); nc.sync.dma_start(rhs_q[:], rhs_q_d[:, :])
        lhs_k = const.tile([P, P], BF16); nc.sync.dma_start(lhs_k[:], lhs_k_d[:, :])
        rhs_v = const.tile([P, P], F32); nc.sync.dma_start(rhs_v[:], rhs_v_d[:, :])
        rhs_o = const.tile([P, P], F32); nc.sync.dma_start(rhs_o[:], rhs_o_d[:, :])
        mh8 = const.tile([P, H], BF16); nc.sync.dma_start(mh8[:], mh8_d[:, :])
        bqc = const.tile([P, 1], F32); nc.sync.dma_start(bqc[:], bqc_d[:, :])
        bkc = const.tile([P, 1], F32); nc.sync.dma_start(bkc[:], bkc_d[:, :])
        brow = {}
        for nm, ap_d in (("bvr", bvr_d), ("bor", bor_d)):
            brow[nm] = const.tile([P, P], F32, name=f"brow_{nm}")
            rep = bass.AP(tensor=ap_d.tensor, offset=ap_d[:, :].offset,
                          ap=[[0, P]] + ap_d[:, :].ap[1:])
            nc.sync.dma_start(brow[nm][:], rep)

        id_f = const.tile([P, P], F32); make_identity(nc, id_f[:])
        distT = const.tile([P, g], F32); nc.sync.dma_start(distT[:], distT_d[:, :])
        hTb_all = const.tile([P, nblk, P], F32)

        x_all = const.tile([P, nblk, P], F32)   # pre-mish outputs

        # ---------------- unified edge + output loop ----------------
        # PSUM: qdps 2 banks + kps 1 + lps 1 + sps 1 + v/aggt/o 3 = 8.
        # Per-block output work (normalize, v/o projections) is interleaved
        # into the edge loop so no phase barrier serializes the program;
        # mish + store go in groups of GS blocks.
        s8_all = const.tile([P, nblk, H], F32)
        GS = 10
        xb = stk.enter_context(tc.tile_pool(name="xb", bufs=4))
        with tc.tile_pool(name="eb", bufs=4) as eb, \
             tc.tile_pool(name="et", bufs=6) as et, \
             tc.tile_pool(name="qdps", bufs=2, space="PSUM") as qdps, \
             tc.tile_pool(name="kps", bufs=2, space="PSUM") as kps, \
             tc.tile_pool(name="lps", bufs=1, space="PSUM") as lps, \
             tc.tile_pool(name="sps", bufs=1, space="PSUM") as sps:
            for b in range(nblk):
                he_b = eb.tile([P, BE], FP8, tag="he")
                nc.sync.dma_start(he_b[:], heT_d[:, ts(b, BE)])
                hs_b = eb.tile([P, BE], FP8, tag="hs")
                nc.sync.dma_start(hs_b[:], hsT_d[:, ts(b, BE)])
                hd_b = eb.tile([P, BE], FP8, tag="hd")
                nc.sync.dma_start(hd_b[:], hdT_d[:, ts(b, BE)])
                u8_b = eb.tile([P, CBLK, P], FP8, tag="u8")
                nc.sync.dma_start(u8_b[:], u8_d[:, ts(b, CBLK * P)])
                ps_s = sps.tile([P, 48], F32, tag="s")
                # projections for tile 0 up front; tiles 1..3 are emitted
                # interleaved into the previous tile's logit matmuls so their
                # LDWEIGHTS hide under the long projection streams
                proj = {}

                def emit_proj(t):
                    qd_ps = qdps.tile([P, 2, TE], F32, tag="qd")
                    nc.tensor.matmul(qd_ps[:, 0, :], rhs_q, hs_b[:, ts(t, TE)])
                    nc.tensor.matmul(qd_ps[:, 1, :], rhs_q, hd_b[:, ts(t, TE)])
                    k_ps = kps.tile([P, TE], F32, tag="k")
                    nc.tensor.matmul(k_ps[:], lhs_k, he_b[:, ts(t, TE)])
                    proj[t] = (qd_ps, k_ps)

                emit_proj(0)
                for t in range(TPB):
                    qd_ps, k_ps = proj.pop(t)

                    qsd = et.tile([P, 2, TE], BF16, tag="qsd")
                    if cfg.get("use_bq"):
                        nc.scalar.activation(qsd[:], qd_ps[:], AF.Identity,
                                             bias=bqc[:, :1])
                    else:
                        nc.scalar.copy(qsd[:], qd_ps[:])

                    # prod0/prod1 = (qs,qd) * k in one op, k read straight
                    # from PSUM with a step-0 broadcast over the pair dim
                    prod = et.tile([P, 3, TE], BF16, tag="prod")
                    if cfg.get("use_bk"):
                        kt = et.tile([P, TE], BF16, tag="kt")
                        nc.scalar.activation(kt[:], k_ps[:], AF.Identity,
                                             bias=bkc[:, :1])
                        k_src = bass.AP(tensor=kt[:].tensor, offset=kt[:].offset,
                                        ap=kt[:].ap[:1] + [[0, 2]] + kt[:].ap[1:])
                    else:
                        k_src = bass.AP(tensor=k_ps[:].tensor, offset=k_ps[:].offset,
                                        ap=k_ps[:].ap[:1] + [[0, 2]] + k_ps[:].ap[1:])
                    nc.vector.tensor_tensor(prod[:, 0:2, :], qsd[:], k_src,
                                            op=OP.mult)
                    nc.gpsimd.tensor_mul(prod[:, 2, :], qsd[:, 0, :], qsd[:, 1, :])

                    # edge-major logits: per 128-edge chunk, stationary = prod;
                    # next tile's projection matmuls are interleaved to hide
                    # the per-chunk LDWEIGHTS under their 512-wide streams
                    ps_l = lps.tile([P, TE // P, 24], F32, tag="l")
                    nxt = []
                    if t + 1 < TPB:
                        qd_n = qdps.tile([P, 2, TE], F32, tag="qd")
                        k_n = kps.tile([P, TE], F32, tag="k")
                        nxt = [lambda: nc.tensor.matmul(qd_n[:, 0, :], rhs_q,
                                                        hs_b[:, ts(t + 1, TE)]),
                               lambda: nc.tensor.matmul(qd_n[:, 1, :], rhs_q,
                                                        hd_b[:, ts(t + 1, TE)]),
                               lambda: nc.tensor.matmul(k_n[:], lhs_k,
                                                        he_b[:, ts(t + 1, TE)])]
                        proj[t + 1] = (qd_n, k_n)
                    for cc in range(TE // P):
                        for j in range(3):
                            nc.tensor.matmul(ps_l[:, cc, 8 * j:8 * (j + 1)],
                                             prod[:, j, ts(cc, P)], mh8[:])
                        if cc < len(nxt):
                            nxt[cc]()

                    # exp and attenuation, edge-major
                    xs = et.tile([P, TE // P, 48], BF16, tag="xs")
                    xs_lo = bass.AP(tensor=xs[:].tensor, offset=xs[:].offset,
                                    ap=xs[:].ap[:1] + [[48, TE // P], [1, 24]])
                    nc.scalar.activation(xs_lo, ps_l[:], AF.Exp, scale=0.25)
                    tg = b * CBLK + t * (TE // P)
                    att = bass.AP(tensor=distT.tensor,
                                  offset=distT[:, tg:tg + TE // P].offset,
                                  ap=distT[:].ap[:1] + [[1, TE // P], [0, 24]])
                    xs_hi = bass.AP(tensor=xs[:].tensor, offset=xs[:].offset + 24,
                                    ap=xs[:].ap[:1] + [[48, TE // P], [1, 24]])
                    nc.gpsimd.tensor_tensor(xs_hi, xs_lo, att, op=OP.mult)

                    # segment sums: one-hot matmul accumulated over the block
                    for cc in range(TE // P):
                        lc = t * (TE // P) + cc
                        nc.tensor.matmul(ps_s[:], u8_b[:, lc, :], xs[:, cc, :],
                                         start=(lc == 0), stop=(lc == CBLK - 1))

                # ---- per-block output work, interleaved ----
                # S[n,h] = sum_t t_sums/(s_sums+eps): reciprocal on ACT,
                # small combines on DVE
                rcp = xb.tile([P, 24], F32, tag="rcp")
                if cfg.get("use_eps"):
                    nc.vector.tensor_scalar_add(rcp[:], ps_s[:, 0:24], EPS)
                    nc.vector.reciprocal(rcp[:], rcp[:])
                else:
                    nc.vector.reciprocal(rcp[:], ps_s[:, 0:24])
                m24 = xb.tile([P, 24], F32, tag="m24")
                nc.vector.tensor_mul(m24[:], ps_s[:, 24:48], rcp[:])
                m24v = bass.AP(tensor=m24[:].tensor, offset=m24[:].offset,
                               ap=[m24[:].ap[0], [1, H], [H, 3]])
                nc.vector.tensor_reduce(s8_all[:, b, :], m24v,
                                        axis=mybir.AxisListType.X, op=OP.add)

        nc.sync.dma_start(hTb_all[:], hTb_d[:, :])
        with tc.tile_pool(name="fb", bufs=6) as fb, \
             tc.tile_pool(name="fps", bufs=2, space="PSUM") as fps:
            for b in range(nblk):
                v_ps = fps.tile([P, 512], F32, tag="v")
                nc.tensor.matmul(v_ps[:, 0:P], hTb_all[:, b, :], rhs_v[:])
                agg = fb.tile([P, P], F32, tag="agg")
                a3 = agg[:].rearrange("p (h d) -> p h d", h=H)
                if cfg.get("use_bv"):
                    v_sb = fb.tile([P, P], F32, tag="vsb")
                    nc.scalar.copy(v_sb[:], v_ps[:, 0:P])
                    nc.vector.tensor_tensor(v_sb[:], v_sb[:], brow["bvr"][:, :],
                                            op=OP.add)
                    v3 = v_sb[:].rearrange("p (h d) -> p h d", h=H)
                else:
                    v3 = v_ps[:, 0:P].rearrange("p (h d) -> p h d", h=H)
                nc.vector.tensor_tensor(a3, v3, bcast(s8_all[:, b, :], DH),
                                        op=OP.mult)

                aggT_ps = fps.tile([P, 512], F32, tag="aggt")
                nc.tensor.transpose(aggT_ps[:, 0:P], agg[:], id_f[:])
                aggT = fb.tile([P, P], F32, tag="aggts")
                nc.scalar.copy(aggT[:], aggT_ps[:, 0:P])

                o_ps = fps.tile([P, 512], F32, tag="o")
                nc.tensor.matmul(o_ps[:, 0:P], aggT[:], rhs_o[:])
                nc.scalar.copy(x_all[:, b, :], o_ps[:, 0:P])
                if cfg.get("use_bo"):
                    nc.vector.tensor_tensor(x_all[:, b, :], x_all[:, b, :],
                                            brow["bor"][:, :], op=OP.add)

            # ---- mish tail: mish(x) = x * tanh(ln(1 + e^x)), staged by
            # function across groups so the ACT table set switches only twice
            u_all = const.tile([P, nblk, P], F32, name="u_all")
            groups = [(g0, min(GS, nblk - g0)) for g0 in range(0, nblk, GS)]
            for g0, gs in groups:
                nc.scalar.activation(u_all[:, g0:g0 + gs, :],
                                     x_all[:, g0:g0 + gs, :], AF.Exp)
            for g0, gs in groups:
                nc.vector.tensor_scalar_add(u_all[:, g0:g0 + gs, :],
                                            u_all[:, g0:g0 + gs, :], 1.0)
            for g0, gs in groups:
                nc.scalar.activation(u_all[:, g0:g0 + gs, :],
                                     u_all[:, g0:g0 + gs, :], AF.Ln)
            for g0, gs in groups:
                nc.scalar.activation(u_all[:, g0:g0 + gs, :],
                                     u_all[:, g0:g0 + gs, :], AF.Tanh)
            for g0, gs in groups:
                o_g = fb.tile([P, GS, P], F32, tag="og")
                nc.vector.tensor_mul(o_g[:, :gs, :], x_all[:, g0:g0 + gs, :],
                                     u_all[:, g0:g0 + gs, :])
                dram = bass.AP(tensor=out_d.tensor, offset=g0 * P * P,
                               ap=[[P, P], [P * P, gs], [1, P]])
                nc.sync.dma_start(dram, o_g[:, :gs, :])

    nc.compile()
    return nc


# ---------------------------------------------------------------- entry
def kernel(**inputs):
    inputs = {k: np.asarray(v) for k, v in inputs.items()}
    cfg, in_maps, meta = _prep(**inputs)

    key = (cfg["nblk"], cfg["use_bq"], cfg["use_bk"], cfg["use_bv"],
           cfg["use_bo"])
    nc = _nc_cache.get(key)
    if nc is None:
        nc = build_program(cfg)
        _nc_cache[key] = nc

    res = bass_utils.run_bass_kernel_spmd(nc, in_maps,
                                          core_ids=list(range(NCORES)))

    n = cfg["n"]
    out = np.zeros((n, D), np.float32)
    for c in range(NCORES):
        oc = res.results[c]["out"]
        for b, (nstart, cnt, _, _) in enumerate(meta[c]):
            out[nstart:nstart + cnt] = oc[b * P:b * P + cnt]
    return out
